# revision 1
# baseline (speedup 1.0000x reference)
"""Trainium2 Bass kernel for nn_AttentionEBM (sparse attention EBM).

Sharding: data-parallel over the batch dim — 32 batches / 8 cores = 4 per core,
processed as 2 pairs stacked along SBUF partitions (batch b in partitions 0:64,
batch b+1 in 64:128) so elementwise engines run at full 128-lane width and the
64-wide matmuls run two-at-a-time via tile_position packing.

Layout: "transposed" everywhere — features on partitions, positions on the free
dim — so each MLP layer is a single matmul with the stored weight matrix as the
stationary operand (out = w.T @ x_T), biases applied as per-partition ACT bias.

Softmax (over 4096 grid / 512 out positions, in [key-part, query-free] layout)
has no cheap per-query max, so a per-query shift M[q] = 3.25*||at_q|| + 12 is
subtracted inside the scores matmul via a rank-1 accumulating matmul
(ones-column x -M row).  The shift only needs to be within ~±80 of the true max
for fp32 exp to be safe; ||at_q|| comes from a ones-matmul of at^2 plus an
integer bit-trick sqrt on the DVE.  The softmax denominator rides as a 65th
ones-column on the value matrix so U^T = [values|1].T @ E yields both the
aggregate and the normalizer in one accumulation.
"""
import numpy as np

RANK, OUT_DIM, N, B, K, H, NF = 64, 512, 4096, 32, 512, 64, 10
NCORES = 8
BPC = B // NCORES          # batches per core
F32 = "float32"

_PROGRAM_CACHE = {}


# ---------------------------------------------------------------- host math
def _posenc(x):
    freqs = 2.0 ** np.arange(NF, dtype=np.float32)
    xf = x[..., None, :] * freqs[:, None]
    sc = np.stack([np.sin(xf), np.cos(xf)], axis=-2)
    return np.concatenate([x, sc.reshape(*x.shape[:-1], -1)], axis=-1)


def _pos_tables():
    ii = np.arange(RANK, dtype=np.float32)
    grid = np.stack(np.meshgrid(ii, ii, indexing="ij"), axis=-1) / RANK
    pos_pe = _posenc(grid).reshape(N, 42)                       # [4096, 42]
    out_pe = _posenc((np.arange(OUT_DIM, dtype=np.float32) / RANK)[:, None])
    return pos_pe, out_pe[:, :21]                               # [512, 21]


def _stack2(a, rows):
    """[rows, C] -> [128, C] with copies at partition 0 and 64."""
    out = np.zeros((128, a.shape[1]), np.float32)
    out[:rows] = a
    out[64:64 + rows] = a
    return out


def _blockdiag(a, rows):
    """[rows, 64] -> [128, 128] block-diagonal: pair-stacked layer in one
    M=128 matmul (fp32r rejects tile_position col offsets)."""
    out = np.zeros((128, 128), np.float32)
    out[0:rows, 0:64] = a
    out[64:64 + rows, 64:128] = a
    return out


def _host_consts(inp):
    pos_pe, out_pe21 = _pos_tables()
    c = {}
    w_lin, b_lin = inp["inp_linear_w"], inp["inp_linear_b"]
    wo_lin, bo_lin = inp["out_linear_w"], inp["out_linear_b"]

    W1 = inp["inp_fc1_w"]
    pe_lhsT = np.concatenate(
        [(W1[:42].T @ w_lin[0])[None], (W1[:42].T @ b_lin)[None], W1[42:84]], 0)
    c["pe_lhsT_s"] = _blockdiag(pe_lhsT, 44)
    c["pe_b1_s"] = _stack2(inp["inp_fc1_b"][:, None], 64)
    c["w2_s"] = _blockdiag(inp["inp_fc2_w"], 64)
    c["b2_s"] = _stack2(inp["inp_fc2_b"][:, None], 64)
    c["w3_s"] = _blockdiag(inp["inp_fc3_w"], 64)

    Wo1 = inp["out_fc1_w"]
    oe_lhsT = np.concatenate(
        [(Wo1[:42].T @ wo_lin[0])[None], (Wo1[:42].T @ bo_lin)[None], Wo1[42:63]], 0)
    c["oe_lhsT_s"] = _blockdiag(oe_lhsT, 23)
    c["oe_b1_s"] = _stack2(inp["out_fc1_b"][:, None], 64)
    c["ow2_s"] = _blockdiag(inp["out_fc2_w"], 64)
    c["ob2_s"] = _stack2(inp["out_fc2_b"][:, None], 64)
    c["ow3_s"] = _blockdiag(inp["out_fc3_w"], 64)

    Wa1 = inp["at_fc1_w"]
    at_lhsT = np.concatenate(
        [(Wa1[:42].T @ w_lin[0])[None], (Wa1[:42].T @ b_lin)[None], Wa1[42:63]], 0)
    c["at_lhsT_s"] = _blockdiag(at_lhsT, 23)
    c["at_b1_s"] = _stack2(inp["at_fc1_b"][:, None], 64)
    c["aw2_s"] = _blockdiag(inp["at_fc2_w"], 64)
    c["ab2_s"] = _stack2(inp["at_fc2_b"][:, None], 64)

    F1 = inp["fc1_w"]
    f1b_eff = (inp["fc1_b"] + F1[64:128].T @ inp["inp_fc3_b"]
               + F1[128:192].T @ inp["out_fc3_b"])
    c["f1a_s"] = _blockdiag(F1[0:64], 64)
    c["f1b_s"] = _blockdiag(F1[64:128], 64)
    c["f1c_s"] = _blockdiag(F1[128:192], 64)
    c["f1b_eff_s"] = _stack2(f1b_eff[:, None], 64)
    c["f2_s"] = _blockdiag(inp["fc2_w"], 64)
    c["f2b_s"] = _stack2(inp["fc2_b"][:, None], 64)
    c["f3_s"] = _stack2(inp["fc3_w"], 64)
    import numpy as _np
    pe_base = _np.zeros((44, N), _np.float32)
    pe_base[1] = 1.0
    pe_base[2:44] = pos_pe.T
    c["pe_base_c"] = pe_base                                    # [44, 4096]
    oe_base = _np.zeros((23, OUT_DIM), _np.float32)
    oe_base[1] = 1.0
    oe_base[2:23] = out_pe21.T
    c["oe_base_c"] = oe_base                                    # [23, 512]
    c["identc"] = np.eye(128, dtype=np.float32)
    c["onesblk"] = np.ones((128, 128), np.float32)
    normones = np.zeros((128, 33), np.float32)
    normones[0:64, 0] = 1.0
    normones[64:128, 32] = 1.0
    c["normones"] = normones
    c["fc3_b"] = float(np.asarray(inp["fc3_b"]).reshape(-1)[0])
    c["pos_pe21"] = pos_pe[:, :21]                              # [4096, 21]
    return c


_CONST_SHAPES = {
    "pe_lhsT_s": (128, 128), "pe_b1_s": (128, 1), "w2_s": (128, 128),
    "b2_s": (128, 1), "w3_s": (128, 128),
    "oe_lhsT_s": (128, 128), "oe_b1_s": (128, 1), "ow2_s": (128, 128),
    "ob2_s": (128, 1), "ow3_s": (128, 128),
    "at_lhsT_s": (128, 128), "at_b1_s": (128, 1), "aw2_s": (128, 128),
    "ab2_s": (128, 1),
    "f1a_s": (128, 128), "f1b_s": (128, 128), "f1c_s": (128, 128),
    "f1b_eff_s": (128, 1), "f2_s": (128, 128), "f2b_s": (128, 1),
    "f3_s": (128, 1),
    "identc": (128, 128), "onesblk": (128, 128), "normones": (128, 33),
    "pe_base_c": (44, 4096), "oe_base_c": (23, 512),
}

ALPHA, BETA = 3.25, 12.0        # softmax shift M = ALPHA*||at|| + BETA
SQRT_MAGIC = 0x1FBD1DF5         # (bits>>1)+magic ~= sqrt, +-3.5%


# ---------------------------------------------------------------- device program
def _build_program(fc3_b, swish_mode="silu", stage=99):
    import concourse.bass as bass
    import concourse.tile as tile
    from concourse import bacc, mybir

    f32, i32 = mybir.dt.float32, mybir.dt.int32
    f32r = mybir.dt.float32r
    Silu = mybir.ActivationFunctionType.Silu
    Exp = mybir.ActivationFunctionType.Exp
    MUL, ADD, SHR = (mybir.AluOpType.mult, mybir.AluOpType.add,
                     mybir.AluOpType.logical_shift_right)

    nc = bacc.Bacc("TRN2", target_bir_lowering=False, debug=False)

    xcore = nc.dram_tensor("xcore", [BPC, OUT_DIM + N], f32r, kind="ExternalInput")
    offs_d = nc.dram_tensor("offs", [BPC, K], i32, kind="ExternalInput")
    pos21_d = nc.dram_tensor("pos21", [BPC, 64, K], f32r, kind="ExternalInput")
    nwcol = sum(s[1] for n, s in _CONST_SHAPES.items()
                if not n.endswith("base_c"))
    wpack_d = nc.dram_tensor("wpack", [128, nwcol], f32r, kind="ExternalInput")
    basepk_d = nc.dram_tensor("basepk", [128, N], f32r, kind="ExternalInput")
    obasepk_d = nc.dram_tensor("obasepk", [128, OUT_DIM], f32r,
                               kind="ExternalInput")
    out_d = nc.dram_tensor("out", [BPC, K], f32, kind="ExternalOutput")
    xflat = xcore[:].rearrange("b n -> (b n)")[:, None]          # [BPC*4608, 1]

    NCH = N // 128      # 32 key chunks
    OCH = OUT_DIM // 128

    lowp = nc.allow_low_precision(reason="float32r is bit-identical fp32 storage")
    with lowp, tile.TileContext(nc) as tc:
        with (
            tc.tile_pool(name="cw", bufs=1) as cw,
            tc.tile_pool(name="big", bufs=1) as big,
            tc.tile_pool(name="sm", bufs=2) as sm,
            tc.tile_pool(name="ep", bufs=4 if swish_mode == "silu" else 3) as ep,
            tc.tile_pool(name="psA", bufs=2, space="PSUM") as psA,
            tc.tile_pool(name="psB", bufs=2, space="PSUM") as psB,
        ):
            # ---- constants in SBUF: one packed tile, W = column slices
            wtile = cw.tile([128, nwcol], f32r, name="wtile")
            nc.sync.dma_start(wtile[:], wpack_d[:])
            W = {}
            col = 0
            for k, s in _CONST_SHAPES.items():
                if k.endswith("base_c"):
                    continue
                W[k] = wtile[:, col:col + s[1]]
                col += s[1]
            ident = W["identc"]
            onesblk = W["onesblk"]
            ones_r = onesblk[0:1, :]

            # ---- persistent per-pair tiles (stage gates for debug)
            if stage < -2:
                dummy = sm.tile([4, 512], f32, name="dummy")
                nc.vector.memset(dummy[:], 0.0)
                nc.sync.dma_start(out_d[:], dummy[:])
            _full_body = stage >= -2
            base = big.tile([128, N], f32r, name="base")
            obase = big.tile([128, OUT_DIM], f32r, name="obase")
            abase = big.tile([128, K], f32r, name="abase")
            nc.sync.dma_start(base[:, :], basepk_d[:])
            nc.sync.dma_start(obase[:, :], obasepk_d[:])

            h1s = big.tile([128, N], f32r, name="h1s")
            speT = big.tile([128, N], f32r, name="speT")
            peb = [big.tile([65, N], f32r, name=f"peb{i}") for i in range(2)]
            oeb = [big.tile([65, OUT_DIM], f32r, name=f"oeb{i}") for i in range(2)]
            atb = [big.tile([65, K], f32r, name=f"atb{i}") for i in range(2)]
            for t in peb:
                nc.sync.dma_start(t[64:65, :], basepk_d[1:2, :])
            for t in oeb + atb:
                nc.sync.dma_start(t[64:65, :], basepk_d[1:2, 0:OUT_DIM])
            pv = [big.tile([128, NCH, 65], f32r, name=f"pv{i}") for i in range(2)]
            ov = [big.tile([128, OCH, 65], f32r, name=f"ov{i}") for i in range(2)]
            for t in pv + ov:
                nc.vector.tensor_copy(t[:, :, 64:65],
                                      onesblk[:, 0:t.shape[1]])
            oeT = big.tile([128, OUT_DIM], f32r, name="oeT")
            soeT = big.tile([128, OUT_DIM], f32r, name="soeT")
            at_st = big.tile([128, K], f32r, name="at_st")
            agg = big.tile([128, K], f32r, name="agg")
            oagg = big.tile([128, K], f32r, name="oagg")
            negM = big.tile([33, K], f32r, name="negM")  # rows 0 / 32 = -M per batch

            HALF = [(slice(0, 64), (0, 0)), (slice(64, 128), (64, 64))]

            f32r = mybir.dt.float32r

            def mm(out, lhsT=None, rhs=None, **kw):
                nc.tensor.matmul(out, lhsT=lhsT.bitcast(f32r),
                                 rhs=rhs.bitcast(f32r), **kw)

            def act_swish(dst, src_ps, bias, width):
                if bias is not None:
                    bias = bias.bitcast(f32)
                """dst = swish(src_ps + bias). silu: 1 ACT op. sigmoid (CoreSim
                debug): ACT sigmoid + DVE bias-add + DVE multiply."""
                if swish_mode == "silu":
                    if bias is None:
                        nc.scalar.activation(dst, src_ps, Silu)
                    else:
                        nc.scalar.activation(dst, src_ps, Silu, bias=bias)
                    return
                Sig = mybir.ActivationFunctionType.Sigmoid
                if bias is None:
                    nc.scalar.activation(dst, src_ps, Sig)
                    tmp = sm.tile([128, width], f32, tag=f"swtmp{width}",
                                  name=f"swtmp{width}")
                    nc.vector.tensor_copy(tmp[:, 0:width], src_ps)
                else:
                    nc.scalar.activation(dst, src_ps, Sig, bias=bias)
                    tmp = sm.tile([128, width], f32, tag=f"swtmp{width}",
                                  name=f"swtmp{width}")
                    nc.vector.tensor_scalar(tmp[:, 0:width], src_ps, bias, None, ADD)
                nc.vector.tensor_tensor(dst, dst, tmp[:, 0:width], MUL)

            def mlp_layer(w_key, b_key, src, dst_act, dst_lin, width, func=Silu,
                          dst_lin_split=None):
                """Pair-stacked K=64 layer over `width` free dim, PSUM groups of 1536.
                dst_act gets func(mm+b) (ACT); dst_lin (optional) gets mm+b (DVE)."""
                gs = 1536
                for g0 in range(0, width, gs):
                    g1 = min(g0 + gs, width)
                    ps = psA.tile([128, 1536], f32, tag="grp")
                    for c0 in range(g0, g1, 512):
                        c1 = min(c0 + 512, g1)
                        mm(ps[:, c0 - g0:c1 - g0], lhsT=W[w_key],
                           rhs=src[:, c0:c1], start=True, stop=True)
                    if dst_act is not None:
                        act_swish(dst_act[:, g0:g1], ps[:, 0:g1 - g0],
                                  W[b_key][:, 0:1], g1 - g0)
                    if dst_lin is not None:
                        nc.vector.tensor_scalar(
                            dst_lin[:, g0:g1], ps[:, 0:g1 - g0],
                            W[b_key][:, 0:1].bitcast(f32), None, ADD)
                    if dst_lin_split is not None:
                        lo, hi = dst_lin_split
                        nc.vector.tensor_scalar(
                            lo[0:64, g0:g1], ps[0:64, 0:g1 - g0],
                            W[b_key][0:64, 0:1].bitcast(f32), None, ADD)
                        nc.vector.tensor_scalar(
                            hi[64:128, g0:g1], ps[64:128, 0:g1 - g0],
                            W[b_key][64:128, 0:1].bitcast(f32), None, ADD)

            # ================= per-pair loop =================
            for p in range(BPC // 2 if stage >= 0 else 0):
                b0, b1 = 2 * p, 2 * p + 1

                # --- input rows
                nc.sync.dma_start(base[0:1, :], xcore[b0:b0 + 1, OUT_DIM:])
                nc.sync.dma_start(base[64:65, :], xcore[b1:b1 + 1, OUT_DIM:])
                nc.sync.dma_start(obase[0:1, :], xcore[b0:b0 + 1, 0:OUT_DIM])
                nc.sync.dma_start(obase[64:65, :], xcore[b1:b1 + 1, 0:OUT_DIM])
                nc.sync.dma_start(abase[0:64, :], pos21_d[b0])
                nc.sync.dma_start(abase[64:128, :], pos21_d[b1])

                if stage < 1:
                    continue
                # --- gather xg[idx] (device gather of x-dependent data)
                for h, b in enumerate((b0, b1)):
                    offs_t = sm.tile([128, 4], i32, tag="offs")
                    nc.sync.dma_start(
                        offs_t[:], offs_d[b].rearrange("(c p) -> p c", p=128))
                    xval = sm.tile([128, 4], f32r, tag="xval")
                    for ch in range(4):
                        nc.gpsimd.indirect_dma_start(
                            out=xval[:, ch:ch + 1], out_offset=None,
                            in_=xflat,
                            in_offset=bass.IndirectOffsetOnAxis(
                                ap=offs_t[:, ch:ch + 1], axis=0))
                    tp_ps = psB.tile([128, 512], f32, tag="bank1")
                    for ch in range(4):
                        mm(
                            tp_ps[0:1, 128 * ch:128 * (ch + 1)],
                            lhsT=xval[:, ch:ch + 1], rhs=ident[:],
                            start=True, stop=True)
                    nc.vector.tensor_copy(
                        abase[64 * h:64 * h + 1, :], tp_ps[0:1, :])

                if stage < 2:
                    continue
                # --- SILU phase: branch MLPs
                mlp_layer("pe_lhsT_s", "pe_b1_s", base, h1s, None, N)
                mlp_layer("w2_s", "b2_s", h1s, speT, None, N,
                          dst_lin_split=(peb[0], h1s))
                nc.sync.dma_start(peb[1][0:64, :], h1s[64:128, :])
                # oe branch (soeT temporarily holds silu(oh1), then oe/soe)
                mlp_layer("oe_lhsT_s", "oe_b1_s", obase, soeT, None, OUT_DIM)
                mlp_layer("ow2_s", "ob2_s", soeT, None, oeT, OUT_DIM)
                nc.sync.dma_start(oeb[0][0:64, :], oeT[0:64, :])
                nc.sync.dma_start(oeb[1][0:64, :], oeT[64:128, :])
                act_swish(soeT[:, :], oeT[:, :], None, OUT_DIM)

                # at branch
                mlp_layer("at_lhsT_s", "at_b1_s", abase, at_st, None, K)
                mlp_layer("aw2_s", "ab2_s", at_st, None, at_st, K)
                nc.sync.dma_start(atb[0][0:64, :], at_st[0:64, :])
                nc.sync.dma_start(atb[1][0:64, :], at_st[64:128, :])

                if stage < 3:
                    continue
                # --- norms -> negM rows
                sq = sm.tile([128, K], f32r, tag="sq")
                nc.vector.tensor_tensor(sq[:], at_st[:], at_st[:], MUL)
                m2ps = psB.tile([128, 512], f32, tag="bank1")
                mm(m2ps[0:33, :], lhsT=W["normones"], rhs=sq[:],
                   start=True, stop=True)
                nmt = sm.tile([33, K], f32, tag="nmt")
                for h in range(2):
                    r = 32 * h
                    nc.vector.tensor_scalar(
                        nmt[r:r + 1, :].bitcast(i32),
                        m2ps[r:r + 1, :].bitcast(i32), 1, None, SHR)
                    nc.vector.tensor_scalar(
                        nmt[r:r + 1, :].bitcast(i32),
                        nmt[r:r + 1, :].bitcast(i32), SQRT_MAGIC, None, ADD)
                    nc.vector.tensor_scalar(negM[r:r + 1, :], nmt[r:r + 1, :],
                                            -ALPHA, -BETA, MUL, ADD)
                    nc.sync.dma_start(atb[h][64:65, :], negM[r:r + 1, :])

                if stage < 4:
                    continue
                # --- pos_val / out_val ([key, 65] layout, col 64 = ones)
                for grp in range(8):            # 4 chunks per PSUM bank
                    pvp = psB.tile([128, 512], f32, tag="bank1", name="pvp")
                    for cc in range(4):
                        ch = grp * 4 + cc
                        mm(pvp[:, 128 * cc:128 * (cc + 1)],
                           lhsT=speT[:, 128 * ch:128 * (ch + 1)],
                           rhs=W["w3_s"], start=True, stop=True)
                    pvv = pvp[:].rearrange("p (c d) -> p c d", c=4)
                    for h in range(2):
                        nc.vector.tensor_copy(
                            pv[h][:, grp * 4:(grp + 1) * 4, 0:64],
                            pvv[:, :, 64 * h:64 * h + 64])
                ovp = psB.tile([128, 512], f32, tag="bank1", name="ovp")
                for ch in range(OCH):
                    mm(ovp[:, 128 * ch:128 * (ch + 1)],
                       lhsT=soeT[:, 128 * ch:128 * (ch + 1)],
                       rhs=W["ow3_s"], start=True, stop=True)
                ovv = ovp[:].rearrange("p (c d) -> p c d", c=4)
                for h in range(2):
                    nc.vector.tensor_copy(ov[h][:, :, 0:64],
                                          ovv[:, :, 64 * h:64 * h + 64])

                if stage < 5:
                    continue
                # --- EXP phase: pos attention, pair-interleaved 3-chunk groups
                U = [psB.tile([65, 512], f32, tag="bank1", name=f"U{h_}") for h_ in range(2)]
                for g0 in range(0, NCH, 3):
                    g1 = min(g0 + 3, NCH)
                    sc = [psA.tile([128, 1536], f32, tag="grp", name=f"sc{h_}") for h_ in range(2)]
                    for ch in range(g0, g1):
                        o0 = 512 * (ch - g0)
                        for h in range(2):
                            mm(sc[h][:, o0:o0 + 512],
                               lhsT=peb[h][:, 128 * ch:128 * (ch + 1)],
                               rhs=atb[h][:, :], start=True, stop=True)
                    for h in range(2):
                        E = ep.tile([128, 1536], f32r, tag="E")
                        w = 512 * (g1 - g0)
                        nc.scalar.activation(E[:, 0:w], sc[h][:, 0:w], Exp)
                        for ch in range(g0, g1):
                            mm(
                                U[h][:, :],
                                lhsT=pv[h][:, ch, :],
                                rhs=E[:, 512 * (ch - g0):512 * (ch - g0 + 1)],
                                start=(ch == 0), stop=(ch == NCH - 1))

                # normalize -> agg
                for h in range(2):
                    Sf = sm.tile([1, K], f32, tag="Sf")
                    nc.vector.tensor_copy(Sf[:], U[h][64:65, :])
                    Rf = sm.tile([1, K], f32, tag="Rf")
                    nc.vector.reciprocal_approx_fast(Rf[:], Sf[:])
                    R = sm.tile([1, K], f32r, tag="R")
                    nc.vector.tensor_copy(R[:], Rf[:])
                    rb = psA.tile([128, 1536], f32, tag="grp")
                    mm(rb[0:64, 0:512], lhsT=ones_r[0:1, 0:64],
                                     rhs=R[:], start=True, stop=True)
                    rbs = sm.tile([64, K], f32, tag="rbs")
                    nc.vector.tensor_copy(rbs[:], rb[0:64, 0:512])
                    nc.vector.tensor_tensor(agg[64 * h:64 * h + 64, :],
                                            U[h][0:64, :], rbs[:], MUL)

                if stage < 6:
                    continue
                # --- out attention
                Uo = [psB.tile([65, 512], f32, tag="bank1", name=f"Uo{h_}") for h_ in range(2)]
                for g0 in range(0, OCH, 2):
                    sc = [psA.tile([128, 1536], f32, tag="grp", name=f"sc{h_}") for h_ in range(2)]
                    for ch in range(g0, g0 + 2):
                        o0 = 512 * (ch - g0)
                        for h in range(2):
                            mm(sc[h][:, o0:o0 + 512],
                               lhsT=oeb[h][:, 128 * ch:128 * (ch + 1)],
                               rhs=atb[h][:, :], start=True, stop=True)
                    for h in range(2):
                        E = ep.tile([128, 1536], f32r, tag="E")
                        nc.scalar.activation(E[:, 0:1024], sc[h][:, 0:1024], Exp)
                        for ch in range(g0, g0 + 2):
                            mm(
                                Uo[h][:, :],
                                lhsT=ov[h][:, ch, :],
                                rhs=E[:, 512 * (ch - g0):512 * (ch - g0 + 1)],
                                start=(ch == 0), stop=(ch == OCH - 1))
                for h in range(2):
                    Sf = sm.tile([1, K], f32, tag="Sf")
                    nc.vector.tensor_copy(Sf[:], Uo[h][64:65, :])
                    Rf = sm.tile([1, K], f32, tag="Rf")
                    nc.vector.reciprocal_approx_fast(Rf[:], Sf[:])
                    R = sm.tile([1, K], f32r, tag="R")
                    nc.vector.tensor_copy(R[:], Rf[:])
                    rb = psA.tile([128, 1536], f32, tag="grp")
                    mm(rb[0:64, 0:512], lhsT=ones_r[0:1, 0:64],
                                     rhs=R[:], start=True, stop=True)
                    rbs = sm.tile([64, K], f32, tag="rbs")
                    nc.vector.tensor_copy(rbs[:], rb[0:64, 0:512])
                    nc.vector.tensor_tensor(oagg[64 * h:64 * h + 64, :],
                                            Uo[h][0:64, :], rbs[:], MUL)

                if stage < 7:
                    continue
                # --- SILU phase: final MLP
                psF = psA.tile([128, 1536], f32, tag="grp")
                for i, (wk, fsrc) in enumerate(
                        [("f1a_s", at_st), ("f1b_s", agg), ("f1c_s", oagg)]):
                    mm(psF[:, 0:512], lhsT=W[wk], rhs=fsrc[:],
                       start=(i == 0), stop=(i == 2))
                fh1 = sm.tile([128, K], f32r, tag="fh1")
                act_swish(fh1[:], psF[:, 0:512], W["f1b_eff_s"][:, 0:1], K)
                psF2 = psA.tile([128, 1536], f32, tag="grp")
                mm(psF2[:, 0:512], lhsT=W["f2_s"], rhs=fh1[:],
                   start=True, stop=True)
                fh2 = sm.tile([128, K], f32r, tag="fh2")
                act_swish(fh2[:], psF2[:, 0:512], W["f2b_s"][:, 0:1], K)
                psO = psB.tile([128, 512], f32, tag="bank1")
                mm(psO[0:1, :], lhsT=W["f3_s"][0:64, 0:1],
                                 rhs=fh2[0:64, :], start=True, stop=True,
                                 tile_position=(0, 0))
                psO2 = psB.tile([128, 512], f32, tag="bank1")
                mm(psO2[0:1, :], lhsT=W["f3_s"][64:128, 0:1],
                                 rhs=fh2[64:128, :], start=True, stop=True,
                                 tile_position=(64, 0))
                for h, pso in enumerate((psO, psO2)):
                    orow = sm.tile([1, K], f32, tag="orow")
                    nc.vector.tensor_scalar(orow[:], pso[0:1, :], fc3_b, None, ADD)
                    nc.sync.dma_start(out_d[2 * p + h:2 * p + h + 1, :], orow[:])

    nc.finalize()
    return nc


# ---------------------------------------------------------------- entry point
def kernel(**inputs) -> np.ndarray:
    from concourse.bass_utils import run_bass_kernel_spmd

    inp = {k: np.asarray(v) for k, v in inputs.items()}
    c = _host_consts(inp)

    key = ("prog", c["fc3_b"])
    if key not in _PROGRAM_CACHE:
        _PROGRAM_CACHE[key] = _build_program(c["fc3_b"])
    nc = _PROGRAM_CACHE[key]

    idx = inp["idx"].astype(np.int64)
    x = inp["x"].astype(np.float32)
    wpack = np.concatenate(
        [c[k] for k in _CONST_SHAPES if not k.endswith("base_c")], axis=1)
    basepk = np.zeros((128, N), np.float32)
    basepk[0:44] = c["pe_base_c"]
    basepk[64:108] = c["pe_base_c"]
    obasepk = np.zeros((128, OUT_DIM), np.float32)
    obasepk[0:23] = c["oe_base_c"]
    obasepk[64:87] = c["oe_base_c"]
    const_arrs = {"wpack": np.ascontiguousarray(wpack, np.float32),
                  "basepk": basepk, "obasepk": obasepk}
    g = c["pos_pe21"][idx].transpose(0, 2, 1)                    # [B, 21, K]
    pos21_all = np.zeros((B, 64, K), np.float32)
    pos21_all[:, 1] = 1.0
    pos21_all[:, 2:23] = g

    in_maps = []
    for core in range(NCORES):
        bs = slice(core * BPC, (core + 1) * BPC)
        local_idx = idx[bs]                                      # [BPC, K]
        offs = (np.arange(BPC)[:, None] * (OUT_DIM + N) + OUT_DIM
                + local_idx).astype(np.int32)
        in_maps.append({
            "xcore": np.ascontiguousarray(x[bs]),
            "offs": np.ascontiguousarray(offs),
            "pos21": np.ascontiguousarray(pos21_all[bs], np.float32),
            **const_arrs,
        })

    res = run_bass_kernel_spmd(nc, in_maps, list(range(NCORES)))
    out = np.concatenate([res.results[core]["out"] for core in range(NCORES)], 0)
    return out.astype(np.float32)


if __name__ == "__main__":
    import pickle
    inp, expected = pickle.load(open("io_cache.pkl", "rb"))
    got = kernel(**inp)
    err = np.abs(got - expected)
    print("max abs err:", err.max(), " rel:", err.max() / np.abs(expected).max())



# revision 40
# speedup vs baseline: 1.4582x; 1.4582x over previous
"""Trainium2 Bass kernel for nn_AttentionEBM (sparse attention EBM).

Sharding: data-parallel over batch — 32 batches / 8 cores = 4 per core,
processed as 2 pairs stacked along SBUF partitions (batch b in partitions
0:64, b+1 in 64:128).

v2 design (from v1 trace analysis: MATMUL 210us of 233us wall, ACT 104us,
PE power-throttled to ~1.2GHz):
 - All PE operands fp16 (weights, activations, score inputs) / bf16
   (exp values, attention values): less PE power -> less DVFS throttle,
   half the SBUF traffic.  PSUM stays fp32.
 - exp() is gone from the ACT engine.  Scores arrive in PSUM already
   M-shifted (rank-1 ones x -M matmul row); a single affine cast
     u16 = round_sat(184.665*(s-M) + 16250.41)
   bit-viewed as bf16 IS e^(s-M) to ~1.5% (Schraudolph).  Float->uint16
   saturation (neg -> 0) gives free underflow flush; bf16's 8-bit
   exponent covers s-M in (-88, +88) and the shift error cancels in the
   softmax ratio.  The cast runs on ACT (Copy w/ scale+bias), DVE and
   Pool (tensor_scalar), weighted 5:4:3, so no single engine serializes
   the softmax.  ACT only ever uses the Silu table -> zero table reloads.
 - Per-chunk score->cast->aggregate pipeline with PSUM 4(sc)+2(U)+2(misc)
   banks so the PE streams back-to-back.
"""
import numpy as np

RANK, OUT_DIM, N, B, K, H, NF = 64, 512, 4096, 32, 512, 64, 10
NCORES = 8
BPC = B // NCORES          # batches per core
NCH = N // 128             # 32 key chunks
OCH = OUT_DIM // 128       # 4 out chunks

_PROGRAM_CACHE = {}

ALPHA, BETA = 3.25, 12.0        # softmax shift M = ALPHA*||at|| + BETA
SQRT_MAGIC = 0x1FBD1DF5         # (bits>>1)+magic ~= sqrt, +-3.5%
EXP_SCALE = 184.6650558         # 2^7 / ln 2
EXP_BIAS = 16250.41             # 127*2^7 - 0.0436776*2^7 (schraudolph, bf16)


# ---------------------------------------------------------------- host math
def _posenc(x):
    freqs = 2.0 ** np.arange(NF, dtype=np.float32)
    xf = x[..., None, :] * freqs[:, None]
    sc = np.stack([np.sin(xf), np.cos(xf)], axis=-2)
    return np.concatenate([x, sc.reshape(*x.shape[:-1], -1)], axis=-1)


def _pos_tables():
    ii = np.arange(RANK, dtype=np.float32)
    grid = np.stack(np.meshgrid(ii, ii, indexing="ij"), axis=-1) / RANK
    pos_pe = _posenc(grid).reshape(N, 42)                       # [4096, 42]
    out_pe = _posenc((np.arange(OUT_DIM, dtype=np.float32) / RANK)[:, None])
    return pos_pe, out_pe[:, :21]                               # [512, 21]


def _blockdiag(a, rows):
    out = np.zeros((128, 128), np.float32)
    out[0:rows, 0:64] = a
    out[64:64 + rows, 64:128] = a
    return out


def _col2(a):
    """[64] -> [128,1] with copies at partition 0 and 64."""
    out = np.zeros((128, 1), np.float32)
    out[0:64, 0] = a
    out[64:128, 0] = a
    return out


_W16_KEYS = ["pe_lhsT", "w2", "w3", "oe_lhsT", "ow2", "ow3",
             "at_lhsT", "aw2", "f1a", "f1b", "f1c", "f2", "identc",
             "normones", "f3"]
_W16_COLS = {k: (1 if k == "f3" else (33 if k == "normones" else 128))
             for k in _W16_KEYS}
_W32_KEYS = ["pe_b1", "b2", "oe_b1", "ob2", "at_b1", "ab2", "f1b_eff", "f2b",
             "onesrow"]
_W32_COLS = {"onesrow": 64}


def _host_consts(inp):
    pos_pe, out_pe21 = _pos_tables()
    c16, c32 = {}, {}
    w_lin, b_lin = inp["inp_linear_w"], inp["inp_linear_b"]
    wo_lin, bo_lin = inp["out_linear_w"], inp["out_linear_b"]

    W1 = inp["inp_fc1_w"]
    pe_lhsT = np.concatenate(
        [(W1[:42].T @ w_lin[0])[None], (W1[:42].T @ b_lin)[None], W1[42:84]], 0)
    c16["pe_lhsT"] = _blockdiag(pe_lhsT, 44)
    c32["pe_b1"] = _col2(inp["inp_fc1_b"])
    c16["w2"] = _blockdiag(inp["inp_fc2_w"], 64)
    c32["b2"] = _col2(inp["inp_fc2_b"])
    c16["w3"] = _blockdiag(inp["inp_fc3_w"], 64)

    Wo1 = inp["out_fc1_w"]
    oe_lhsT = np.concatenate(
        [(Wo1[:42].T @ wo_lin[0])[None], (Wo1[:42].T @ bo_lin)[None], Wo1[42:63]], 0)
    c16["oe_lhsT"] = _blockdiag(oe_lhsT, 23)
    c32["oe_b1"] = _col2(inp["out_fc1_b"])
    c16["ow2"] = _blockdiag(inp["out_fc2_w"], 64)
    c32["ob2"] = _col2(inp["out_fc2_b"])
    c16["ow3"] = _blockdiag(inp["out_fc3_w"], 64)

    Wa1 = inp["at_fc1_w"]
    at_lhsT = np.concatenate(
        [(Wa1[:42].T @ w_lin[0])[None], (Wa1[:42].T @ b_lin)[None], Wa1[42:63]], 0)
    c16["at_lhsT"] = _blockdiag(at_lhsT, 23)
    c32["at_b1"] = _col2(inp["at_fc1_b"])
    c16["aw2"] = _blockdiag(inp["at_fc2_w"], 64)
    c32["ab2"] = _col2(inp["at_fc2_b"])

    F1 = inp["fc1_w"]
    f1b_eff = (inp["fc1_b"] + F1[64:128].T @ inp["inp_fc3_b"]
               + F1[128:192].T @ inp["out_fc3_b"])
    c16["f1a"] = _blockdiag(F1[0:64], 64)
    c16["f1b"] = _blockdiag(F1[64:128], 64)
    c16["f1c"] = _blockdiag(F1[128:192], 64)
    c32["f1b_eff"] = _col2(f1b_eff)
    c16["f2"] = _blockdiag(inp["fc2_w"], 64)
    c32["f2b"] = _col2(inp["fc2_b"])
    f3 = np.zeros((128, 1), np.float32)
    f3[0:64] = inp["fc3_w"]
    f3[64:128] = inp["fc3_w"]
    c16["f3"] = f3
    c16["identc"] = np.eye(128, dtype=np.float32)

    normones = np.zeros((128, 33), np.float32)
    normones[0:64, 0] = 1.0
    normones[64:128, 32] = 1.0
    c16["normones"] = normones
    onesrow = np.zeros((128, 64), np.float32)
    onesrow[0, :] = 1.0
    c32["onesrow"] = onesrow

    # base tables (fp16): row0 xg (per pair), row1 ones, rows 2-43 pos_pe^T
    basepk = np.zeros((128, N), np.float32)
    basepk[1] = 1.0
    basepk[2:44] = pos_pe.T
    basepk[65] = 1.0
    basepk[66:108] = pos_pe.T
    obasepk = np.zeros((128, OUT_DIM), np.float32)
    obasepk[1] = 1.0
    obasepk[2:23] = out_pe21.T
    obasepk[65] = 1.0
    obasepk[66:87] = out_pe21.T

    wpack16 = np.concatenate([c16[k] for k in _W16_KEYS], axis=1)
    wpack32 = np.concatenate([c32[k] for k in _W32_KEYS], axis=1)
    return {
        "wpack16": wpack16.astype(np.float16),
        "wpack32": wpack32.astype(np.float32),
        "basepk": basepk.astype(np.float16),
        "obasepk": obasepk.astype(np.float16),
        "fc3_b": float(np.asarray(inp["fc3_b"]).reshape(-1)[0]),
        "pos_pe21": pos_pe[:, :21],
    }


# ---------------------------------------------------------------- device program
def _build_program(fc3_b, dbg=False):
    import concourse.bass as bass
    import concourse.tile as tile
    from concourse import bacc, mybir

    f32, i32, u16 = mybir.dt.float32, mybir.dt.int32, mybir.dt.uint16
    f16, bf16 = mybir.dt.float16, mybir.dt.bfloat16
    f32r = mybir.dt.float32r
    Silu = mybir.ActivationFunctionType.Silu
    Copy = mybir.ActivationFunctionType.Copy
    MUL, ADD, SHR = (mybir.AluOpType.mult, mybir.AluOpType.add,
                     mybir.AluOpType.logical_shift_right)

    nc = bacc.Bacc("TRN2", target_bir_lowering=False, debug=False)

    xcore = nc.dram_tensor("xcore", [BPC, OUT_DIM + N], f16, kind="ExternalInput")
    offs_d = nc.dram_tensor("offs", [BPC, K], i32, kind="ExternalInput")
    pos21_d = nc.dram_tensor("pos21", [BPC, 22, K], f16, kind="ExternalInput")
    n16 = sum(_W16_COLS[k] for k in _W16_KEYS)
    n32 = sum(_W32_COLS.get(k, 1) for k in _W32_KEYS)
    wpack16_d = nc.dram_tensor("wpack16", [128, n16], f16, kind="ExternalInput")
    wpack32_d = nc.dram_tensor("wpack32", [128, n32], f32r, kind="ExternalInput")
    basepk_d = nc.dram_tensor("basepk", [128, N], f16, kind="ExternalInput")
    obasepk_d = nc.dram_tensor("obasepk", [128, OUT_DIM], f16,
                               kind="ExternalInput")
    out_d = nc.dram_tensor("out", [BPC, K], f32, kind="ExternalOutput")
    xflat = xcore[:].rearrange("b n -> (b n)")[:, None]          # [BPC*4608, 1]
    if dbg:
        dbg_d = {name: nc.dram_tensor(f"dbg_{name}", shape, dt, kind="ExternalOutput")
                 for name, shape, dt in [
                     ("abase", [128, K], f16), ("atb0", [65, K], f16),
                     ("E0", [128, K], bf16), ("E1", [128, K], bf16),
                     ("U0", [65, K], f32), ("rbs0", [64, K], f32),
                     ("peb0", [65, 512], f16), ("h1s", [128, 512], f16),
                     ("speT", [128, 512], f16), ("pv0", [128, 130], bf16),
                     ("agg", [128, K], f16), ("oagg", [128, K], f16),
                     ("oeb0", [65, OUT_DIM], f16), ("at_st", [128, K], f16)]}

    lowp = nc.allow_low_precision(reason="fp16/bf16 attention path validated "
                                  "against reference in numpy (rel 4.8e-3)")
    with lowp, tile.TileContext(nc) as tc:
        with (
            tc.tile_pool(name="cw", bufs=1) as cw,
            tc.tile_pool(name="big", bufs=1) as big,
            tc.tile_pool(name="sm", bufs=2) as sm,
            tc.tile_pool(name="ep", bufs=4) as ep,
            tc.tile_pool(name="psS", bufs=4, space="PSUM") as psS,
            tc.tile_pool(name="psU", bufs=2, space="PSUM") as psU,
            tc.tile_pool(name="psM", bufs=2, space="PSUM") as psM,
        ):
            # ---- constants
            wt16 = cw.tile([128, n16], f16, name="wt16")
            wt32 = cw.tile([128, n32], f32r, name="wt32")
            nc.scalar.dma_start(wt16[:], wpack16_d[:])
            nc.scalar.dma_start(wt32[:], wpack32_d[:])
            W16, W32 = {}, {}
            col = 0
            for k in _W16_KEYS:
                w = _W16_COLS[k]
                W16[k] = wt16[:, col:col + w]
                col += w
            col = 0
            for k in _W32_KEYS:
                w = _W32_COLS.get(k, 1)
                W32[k] = wt32[:, col:col + w]
                col += w

            # ---- persistent tiles
            base = big.tile([128, N], f16, name="base")
            obase = big.tile([128, OUT_DIM], f16, name="obase")
            abase = big.tile([128, K], f16, name="abase")
            # rows 23-63 / 87-127 of abase are multiplied by zero lhsT rows
            # but must not hold NaN garbage (0 * NaN = NaN in the PE)
            nc.vector.memset(abase[:, :], 0.0)
            # column-chunked so pe-L1 c0 doesn't wait for the full 1MB
            # table transfer, just its own 512 columns
            for c0 in range(0, N, 512):
                nc.gpsimd.dma_start(base[:, c0:c0 + 512],
                                    basepk_d[:, c0:c0 + 512])
            nc.gpsimd.dma_start(obase[:, :], obasepk_d[:])

            h1s = big.tile([128, N], f16, name="h1s")
            speT = big.tile([128, N], f16, name="speT")
            soeT = big.tile([128, OUT_DIM], f16, name="soeT")
            oestage = big.tile([128, OUT_DIM], f16, name="oestage")
            at_st = big.tile([128, K], f16, name="at_st")
            peb = [big.tile([65, N], f16, name=f"peb{i}") for i in range(2)]
            oeb = [big.tile([65, OUT_DIM], f16, name=f"oeb{i}") for i in range(2)]
            atb = [big.tile([65, K], f16, name=f"atb{i}") for i in range(2)]
            for t in peb:
                nc.sync.dma_start(t[64:65, :], basepk_d[1:2, :])
            for t in oeb:
                nc.sync.dma_start(t[64:65, :], basepk_d[1:2, 0:OUT_DIM])
            pv = [big.tile([128, NCH, 65], bf16, name=f"pv{i}") for i in range(2)]
            ov = [big.tile([128, OCH, 65], bf16, name=f"ov{i}") for i in range(2)]
            for t in pv + ov:
                nc.vector.memset(t[:, :, 64:65], 1.0)
            agg = big.tile([128, K], f16, name="agg")
            oagg = big.tile([128, K], f16, name="oagg")
            negM16 = big.tile([33, K], f16, name="negM16")

            onesr = W32["onesrow"][0:1, :]          # [1, 64] f32 row of ones

            def mmr(out, lhsT, rhs, **kw):
                nc.tensor.matmul(out, lhsT=lhsT.bitcast(f32r),
                                 rhs=rhs.bitcast(f32r), **kw)

            # alternating cast engines (ACT / DVE; Pool cannot read PSUM)
            cast_state = [0]

            def exp_cast(E_bf16_ap, sc_ps):
                """E = schraudolph-exp(sc) via saturating f32->u16 affine cast."""
                eng = cast_state[0] % 2
                cast_state[0] += 1
                ev = E_bf16_ap.bitcast(u16)
                if eng == 0:
                    nc.scalar.activation(ev, sc_ps, Copy,
                                         bias=EXP_BIAS, scale=EXP_SCALE)
                else:
                    nc.vector.tensor_scalar(ev, sc_ps, EXP_SCALE, EXP_BIAS,
                                            MUL, ADD)

            scr_state = [0]

            def scratch_tile(name):
                """Alternate scratch PSUM between psM(2) and psS(4) so MLP
                chunk N never WAR-waits on silu of chunk N-2."""
                scr_state[0] += 1
                if scr_state[0] % 2 == 0:
                    return psM.tile([128, 512], f32, tag="m", name=name)
                return psS.tile([128, 512], f32, tag="sc", name=name)

            def mlp_layer(w_key, b_key, src, dst_act, dst_lin, width):
                """Pair-stacked layer, 512-wide PSUM chunks."""
                for c0 in range(0, width, 512):
                    ps = scratch_tile("mlp_ps")
                    nc.tensor.matmul(ps[:, :], lhsT=W16[w_key],
                                     rhs=src[:, c0:c0 + 512],
                                     start=True, stop=True)
                    if dst_act is not None:
                        nc.scalar.activation(dst_act[:, c0:c0 + 512], ps[:, :],
                                             Silu,
                                             bias=W32[b_key][:, 0:1].bitcast(f32))
                    if dst_lin is not None:
                        nc.vector.tensor_scalar(
                            dst_lin[:, c0:c0 + 512], ps[:, :],
                            W32[b_key][:, 0:1].bitcast(f32), None, ADD)


            def attention(src_b, val, Upool_tag, nch, dst, dbg_cap=False):
                """softmax(at . src) @ val for both halves; dst fp16 [128,K]."""
                U = [psM.tile([128, 512], f32, tag="m", name=f"U{h}")
                     for h in range(2)]
                for ch in range(nch):
                    for h in range(2):
                        sc = psS.tile([128, 512], f32, tag="sc", name="sc")
                        nc.tensor.matmul(
                            sc[:, :], lhsT=src_b[h][:, 128 * ch:128 * (ch + 1)],
                            rhs=atb[h][:, :], start=True, stop=True)
                        E = ep.tile([128, 512], bf16, tag=f"E{h}", name="E")
                        exp_cast(E[:, :], sc[:, :])
                        if dbg_cap and h == 0 and ch < 2:
                            nc.sync.dma_start(dbg_d[f"E{ch}"][:], E[:, :])
                        nc.tensor.matmul(
                            U[h][0:65, :], lhsT=val[h][:, ch, 0:65],
                            rhs=E[:, :], start=(ch == 0), stop=(ch == nch - 1))
                if dbg_cap:
                    u0s = sm.tile([65, K], f32, tag="u0s", name="u0s")
                    nc.vector.tensor_copy(u0s[:, :], U[0][0:65, :])
                    nc.sync.dma_start(dbg_d["U0"][:], u0s[:, :])
                for h in range(2):
                    Sf = sm.tile([1, K], f32, tag="Sf", name="Sf")
                    nc.scalar.activation(Sf[:, :], U[h][64:65, :], Copy)
                    Rf = sm.tile([1, K], f32, tag="Rf", name="Rf")
                    nc.vector.reciprocal_approx_fast(Rf[:], Sf[:, :])
                    R = sm.tile([1, K], f32r, tag="R", name="R")
                    nc.vector.tensor_copy(R[:], Rf[:])
                    rb = psM.tile([128, 512], f32, tag="m", name="rb")
                    mmr(rb[0:64, :], onesr, R[:], start=True, stop=True)
                    rbs = sm.tile([64, K], f32, tag="rbs", name="rbs")
                    nc.scalar.activation(rbs[:, :], rb[0:64, :], Copy)
                    if dbg_cap and h == 0:
                        nc.sync.dma_start(dbg_d["rbs0"][:], rbs[:, :])
                    nc.vector.tensor_tensor(dst[64 * h:64 * h + 64, :],
                                            U[h][0:64, :], rbs[:, :], MUL)

            # ================= per-pair phases =================
            def prefetch_dma(p):
                """Input DMAs + idx gathers for pair p (no PE work).
                offs first: the Pool gathers depend only on it + DRAM."""
                b0, b1 = 2 * p, 2 * p + 1
                nc.sync.dma_start(base[0:1, :], xcore[b0:b0 + 1, OUT_DIM:])
                nc.sync.dma_start(base[64:65, :], xcore[b1:b1 + 1, OUT_DIM:])
                nc.sync.dma_start(obase[0:1, :], xcore[b0:b0 + 1, 0:OUT_DIM])
                nc.sync.dma_start(obase[64:65, :], xcore[b1:b1 + 1, 0:OUT_DIM])
                xvals = []
                for h, b in enumerate((b0, b1)):
                    offs_t = sm.tile([128, 4], i32, tag="offs", name="offs_t")
                    nc.sync.dma_start(
                        offs_t[:], offs_d[b].rearrange("(c p) -> p c", p=128))
                    xval = sm.tile([128, 4], f16, tag="xval", name="xval")
                    for ch in range(4):
                        nc.gpsimd.indirect_dma_start(
                            out=xval[:, ch:ch + 1], out_offset=None,
                            in_=xflat,
                            in_offset=bass.IndirectOffsetOnAxis(
                                ap=offs_t[:, ch:ch + 1], axis=0))
                    xvals.append(xval)
                nc.gpsimd.dma_start(abase[1:23, :], pos21_d[b0])
                nc.gpsimd.dma_start(abase[65:87, :], pos21_d[b1])
                return xvals

            def prefetch_pe_a(p, xvals):
                """Gather transpose + at L1 for pair p."""
                at_st = at_stD[p % 2]
                for h in range(2):
                    tp_ps = psM.tile([128, 512], f32, tag="m", name="tp_ps")
                    for ch in range(4):
                        nc.tensor.matmul(
                            tp_ps[0:1, 128 * ch:128 * (ch + 1)],
                            lhsT=xvals[h][:, ch:ch + 1], rhs=W16["identc"],
                            start=True, stop=True)
                    nc.vector.tensor_copy(
                        abase[64 * h:64 * h + 1, :], tp_ps[0:1, :])
                mlp_layer("at_lhsT", "at_b1", abase, at_st, None, K)

            def prefetch_pe_b(p):
                """at L2 + norms -> atb rows for pair p."""
                at_st = at_stD[p % 2]
                atb = atbD[p % 2]
                mlp_layer("aw2", "ab2", at_st, None, at_st, K)
                nc.sync.dma_start(atb[0][0:64, :], at_st[0:64, :])
                nc.sync.dma_start(atb[1][0:64, :], at_st[64:128, :])
                sq = sm.tile([128, K], f16, tag="sq", name="sq")
                nc.vector.tensor_tensor(sq[:], at_st[:], at_st[:], MUL)
                m2 = psM.tile([128, 512], f32, tag="m", name="m2")
                nc.tensor.matmul(m2[0:33, :], lhsT=W16["normones"], rhs=sq[:],
                                 start=True, stop=True)
                nmt = sm.tile([33, K], f32, tag="nmt", name="nmt")
                nc.vector.tensor_scalar(nmt[0:33, :].bitcast(i32),
                                        m2[0:33, :].bitcast(i32), 1, None, SHR)
                nc.vector.tensor_scalar(nmt[0:33, :].bitcast(i32),
                                        nmt[0:33, :].bitcast(i32),
                                        SQRT_MAGIC, None, ADD)
                nc.vector.tensor_scalar(negM16[0:33, :], nmt[0:33, :],
                                        -ALPHA, -BETA, MUL, ADD)
                nc.sync.dma_start(atb[0][64:65, :], negM16[0:1, :])
                nc.sync.dma_start(atb[1][64:65, :], negM16[32:33, :])

            def pair_pe_l1(p):
                mlp_layer("pe_lhsT", "pe_b1", base, h1s, None, N)

            def pair_mlps(p, xv=None):
                """pe L2, oe branch, values; the NEXT-attention pair's
                at-branch (xv = its gathered xg) is interleaved so its
                serial PE->ACT->PE->DVE chain overlaps the values phase."""
                peb, oeb = pebD[p % 2], oebD[p % 2]
                mlp_layer("w2", "b2", h1s, speT, h1s, N)
                nc.gpsimd.dma_start(peb[0][0:64, :], h1s[0:64, :])
                nc.gpsimd.dma_start(peb[1][0:64, :], h1s[64:128, :])
                if xv is not None:
                    prefetch_pe_a(p, xv)
                mlp_layer("oe_lhsT", "oe_b1", obase, soeT, None, OUT_DIM)
                mlp_layer("ow2", "ob2", soeT, soeT, oestage, OUT_DIM)
                nc.gpsimd.dma_start(oeb[0][0:64, :], oestage[0:64, :])
                nc.gpsimd.dma_start(oeb[1][0:64, :], oestage[64:128, :])
                for grp in range(8):
                    if grp == 4 and xv is not None:
                        prefetch_pe_b(p)
                    pvp = scratch_tile("pvp")
                    for cc in range(4):
                        ch = grp * 4 + cc
                        nc.tensor.matmul(
                            pvp[:, 128 * cc:128 * (cc + 1)],
                            lhsT=speT[:, 128 * ch:128 * (ch + 1)],
                            rhs=W16["w3"], start=True, stop=True)
                    pvv = pvp[:].rearrange("p (c d) -> p c d", c=4)
                    nc.vector.tensor_copy(
                        pv[0][:, grp * 4:(grp + 1) * 4, 0:64],
                        pvv[:, :, 0:64])
                    nc.scalar.activation(
                        pv[1][:, grp * 4:(grp + 1) * 4, 0:64],
                        pvv[:, :, 64:128], Copy)
                ovp = scratch_tile("ovp")
                for ch in range(OCH):
                    nc.tensor.matmul(
                        ovp[:, 128 * ch:128 * (ch + 1)],
                        lhsT=soeT[:, 128 * ch:128 * (ch + 1)],
                        rhs=W16["ow3"], start=True, stop=True)
                ovv = ovp[:].rearrange("p (c d) -> p c d", c=4)
                nc.vector.tensor_copy(ov[0][:, :, 0:64], ovv[:, :, 0:64])
                nc.scalar.activation(ov[1][:, :, 0:64], ovv[:, :, 64:128], Copy)

            def pair_attention(p):
                """Software-pipelined: U-accum for chunk ch-1 issued after
                scores for chunk ch so the in-order PE never waits on casts."""
                peb, oeb, atb = pebD[p % 2], oebD[p % 2], atbD[p % 2]

                def score_pair(src_b, ch):
                    Es = []
                    for h in range(2):
                        sc = psS.tile([128, 512], f32, tag="sc", name="sc")
                        nc.tensor.matmul(
                            sc[:, :],
                            lhsT=src_b[h][:, 128 * ch:128 * (ch + 1)],
                            rhs=atb[h][:, :], start=True, stop=True)
                        E = ep.tile([128, 512], bf16, tag=f"E{h}", name="E")
                        exp_cast(E[:, :], sc[:, :])
                        Es.append(E)
                    return Es

                def accum_pair(U, val, ch, nch, Es):
                    for h in range(2):
                        nc.tensor.matmul(
                            U[h][0:65, :], lhsT=val[h][:, ch, 0:65],
                            rhs=Es[h][:, :],
                            start=(ch == 0), stop=(ch == nch - 1))

                def normalize(U, h, dst):
                    Sf = sm.tile([1, K], f32, tag="Sf", name="Sf")
                    nc.scalar.activation(Sf[:, :], U[h][64:65, :], Copy)
                    Rf = sm.tile([1, K], f32, tag="Rf", name="Rf")
                    nc.vector.reciprocal_approx_fast(Rf[:], Sf[:, :])
                    rbb = sm.tile([64, K], f32, tag="rbb", name="rbb")
                    nc.gpsimd.partition_broadcast(rbb[:, :], Rf[:, :])
                    nc.vector.tensor_tensor(dst[64 * h:64 * h + 64, :],
                                            U[h][0:64, :], rbb[:, :], MUL)

                Up = [psU.tile([128, 512], f32, tag="U", name=f"Up{h}")
                      for h in range(2)]
                Es = score_pair(peb, 0)
                for ch in range(1, NCH):
                    Es_new = score_pair(peb, ch)
                    accum_pair(Up, pv, ch - 1, NCH, Es)
                    Es = Es_new
                accum_pair(Up, pv, NCH - 1, NCH, Es)

                Eo0 = score_pair(oeb, 0)
                Eo1 = score_pair(oeb, 1)
                for h in range(2):
                    normalize(Up, h, agg)
                Uo = [psU.tile([128, 512], f32, tag="U", name=f"Uo{h}")
                      for h in range(2)]
                Eo2 = score_pair(oeb, 2)
                accum_pair(Uo, ov, 0, OCH, Eo0)
                Eo3 = score_pair(oeb, 3)
                accum_pair(Uo, ov, 1, OCH, Eo1)
                accum_pair(Uo, ov, 2, OCH, Eo2)
                accum_pair(Uo, ov, 3, OCH, Eo3)
                for h in range(2):
                    normalize(Uo, h, oagg)
                if dbg and p == 0:
                    nc.sync.dma_start(dbg_d["abase"][:], abase[:, :])
                    nc.sync.dma_start(dbg_d["atb0"][:], atb[0][:, :])
                    nc.sync.dma_start(dbg_d["agg"][:], agg[:, :])
                    nc.sync.dma_start(dbg_d["oagg"][:], oagg[:, :])

            def pair_final(p, interleave=None):
                """Final MLP.  `interleave` (a list of thunks issuing one PE
                matmul each) is drained between stages to hide the ACT
                silu round-trips."""
                at_st = at_stD[p % 2]
                il = list(interleave or [])

                def drain(k):
                    for _ in range(k):
                        if il:
                            il.pop(0)()

                drain(5)
                psF = psM.tile([128, 512], f32, tag="m", name="psF")
                for i, (wk, fsrc) in enumerate(
                        [("f1a", at_st), ("f1b", agg), ("f1c", oagg)]):
                    nc.tensor.matmul(psF[:, :], lhsT=W16[wk], rhs=fsrc[:],
                                     start=(i == 0), stop=(i == 2))
                drain(3)
                fh1 = sm.tile([128, K], f16, tag="fh1", name="fh1")
                nc.scalar.activation(fh1[:], psF[:, :], Silu,
                                     bias=W32["f1b_eff"][:, 0:1].bitcast(f32))
                psF2 = psM.tile([128, 512], f32, tag="m", name="psF2")
                nc.tensor.matmul(psF2[:, :], lhsT=W16["f2"], rhs=fh1[:],
                                 start=True, stop=True)
                drain(3)
                fh2 = sm.tile([128, K], f16, tag="fh2", name="fh2")
                nc.scalar.activation(fh2[:], psF2[:, :], Silu,
                                     bias=W32["f2b"][:, 0:1].bitcast(f32))
                for h in range(2):
                    psO = psS.tile([128, 512], f32, tag="sc", name="psO")
                    nc.tensor.matmul(psO[0:1, :],
                                     lhsT=W16["f3"][64 * h:64 * h + 64, 0:1],
                                     rhs=fh2[64 * h:64 * h + 64, :],
                                     start=True, stop=True,
                                     tile_position=(64 * h, 0))
                    orow = sm.tile([1, K], f32, tag="orow", name="orow")
                    nc.vector.tensor_scalar(orow[:], psO[0:1, :], fc3_b,
                                            None, ADD)
                    nc.sync.dma_start(out_d[2 * p + h:2 * p + h + 1, :],
                                      orow[:])
                drain(99)

            def pe_l1_thunks(p):
                """pe L1 for pair p as a list of single-matmul thunks (for
                interleaving into the previous pair's final MLP)."""
                thunks = []
                for c0 in range(0, N, 512):
                    def t(c0=c0):
                        ps = scratch_tile("mlp_ps")
                        nc.tensor.matmul(ps[:, :], lhsT=W16["pe_lhsT"],
                                         rhs=base[:, c0:c0 + 512],
                                         start=True, stop=True)
                        nc.scalar.activation(
                            h1s[:, c0:c0 + 512], ps[:, :], Silu,
                            bias=W32["pe_b1"][:, 0:1].bitcast(f32))
                    thunks.append(t)
                return thunks

            # ================= schedule =================
            # final(p) is issued AFTER pair p+1's MLPs: the ~17us MLP/values
            # PE stream hides the serial normalize chain (ACT/DVE/Pool) that
            # produces agg/oagg, so psF never stalls the PE.
            NPAIR = BPC // 2
            xv = prefetch_dma(0)
            for t in pe_l1_thunks(0):
                t()
            pair_mlps(0, xv=xv)
            for p in range(NPAIR):
                if p + 1 < NPAIR:
                    xv = prefetch_dma(p + 1)
                pair_attention(p)
                if p + 1 < NPAIR:
                    for t in pe_l1_thunks(p + 1):
                        t()
                    pair_mlps(p + 1, xv=xv)
                pair_final(p)

    nc.finalize()
    return nc


# ---------------------------------------------------------------- host entry
def _prepare(inp):
    """Build (cached) program + per-core input maps from full inputs."""
    inp = {k: np.asarray(v) for k, v in inp.items()}
    c = _host_consts(inp)
    key = ("prog", c["fc3_b"])
    if key not in _PROGRAM_CACHE:
        _PROGRAM_CACHE[key] = _build_program(c["fc3_b"])
    nc = _PROGRAM_CACHE[key]

    idx = inp["idx"].astype(np.int64)
    x16 = inp["x"].astype(np.float16)                            # [B, 4608]
    g = c["pos_pe21"][idx].transpose(0, 2, 1)                    # [B, 21, K]
    pos21 = np.zeros((B, 22, K), np.float16)
    pos21[:, 0] = 1.0                                            # ones row
    pos21[:, 1:22] = g.astype(np.float16)
    const_arrs = {"wpack16": np.ascontiguousarray(c["wpack16"]),
                  "wpack32": np.ascontiguousarray(c["wpack32"]),
                  "basepk": np.ascontiguousarray(c["basepk"]),
                  "obasepk": np.ascontiguousarray(c["obasepk"])}
    in_maps = []
    for core in range(NCORES):
        bs = slice(core * BPC, (core + 1) * BPC)
        local_idx = idx[bs]
        offs = (np.arange(BPC)[:, None] * (OUT_DIM + N) + OUT_DIM
                + local_idx).astype(np.int32)
        in_maps.append({
            "xcore": np.ascontiguousarray(x16[bs]),
            "offs": np.ascontiguousarray(offs),
            "pos21": np.ascontiguousarray(pos21[bs]),
            **const_arrs,
        })
    return nc, in_maps


def kernel(**inputs) -> np.ndarray:
    from concourse.bass_utils import run_bass_kernel_spmd

    nc, in_maps = _prepare(inputs)
    res = run_bass_kernel_spmd(nc, in_maps, list(range(NCORES)))
    out = np.concatenate([res.results[core]["out"] for core in range(NCORES)], 0)
    return out.astype(np.float32)


if __name__ == "__main__":
    import pickle
    inp, expected = pickle.load(open("io_cache.pkl", "rb"))
    got = kernel(**inp)
    err = np.abs(got - expected)
    print("max abs err:", err.max(), " rel:", err.max() / np.abs(expected).max())
